# revision 67
# baseline (speedup 1.0000x reference)
"""MAB (multihead attention block) TRN2 kernel.

Sharding: 8 cores = batch (4) x query-half (2). Each core computes its
[1024, 256] output slice with zero cross-core communication (K/V
projections are recomputed by the 2 cores sharing a batch).

Layout strategy: everything transposed (features on partitions) so that
- projections contract d on partitions,
- scores come out as [k, q] (exp output directly usable as A@V rhs),
- softmax denominators via a ones-column appended to each head's V
  block (the A@V matmul computes the row-sum for free in PSUM row 64),
- LN stats via ones-vector matmuls; row broadcasts on the gpsimd engine,
- FFN contracts e on partitions directly.

I/O is DMA'd in natural layout (1KB contiguous rows) and transposed on
the PE with an identity matmul; 4-byte-element DMA transposes cost
~7ns/element and would dominate.

The emission order is hand-interleaved so that
- the first attention block starts as soon as K chunk 0 is projected
  (load chunks stream through transpose->project->attention),
- each block's softmax normalization is deferred past the next block's
  matmul loop (hides the denominator DMA round trip),
- LN/FFN work for query-block 0 is injected into the last two attention
  blocks' instruction streams (runs on DVE/Pool/ACT slack),
- one ACT LUT set (Exp+Ln+Square+Copy) serves the whole kernel.

All matmuls run in float32r (~1.3e-4 rel err, full PE rate).
"""

import numpy as np

import concourse.bass as bass
import concourse.mybir as mybir
import concourse.tile as tile
from concourse import bacc
from concourse import masks
from concourse.bass_utils import run_bass_kernel_spmd

F32 = mybir.dt.float32
F32R = mybir.dt.float32r
AF = mybir.ActivationFunctionType
ALU = mybir.AluOpType

B, NQ, NK, D = 4, 2048, 2048, 256
H, DH = 4, 64
S = NQ // 2          # queries per core
ET = D // 128        # feature tiles
QB = S // 512        # query blocks of 512
KT = NK // 128       # key tiles of 128
KB = NK // 512       # key blocks of 512
EPS = 1e-5
SCALE = 1.0 / np.sqrt(D)

_CACHE = {}


def _build(flags):
    (use_bq, use_bk, use_bv, use_bo, use_g0, use_g1) = flags
    nc = bacc.Bacc(None, target_bir_lowering=False)

    dQ = nc.dram_tensor("Qs", [S, D], F32, kind="ExternalInput")
    dK = nc.dram_tensor("Ks", [NK, D], F32, kind="ExternalInput")
    dW = {w: nc.dram_tensor(w, [D, D], F32, kind="ExternalInput")
          for w in ("Wq", "Wk", "Wv", "Wo")}
    dV = {v: nc.dram_tensor(v, [D], F32, kind="ExternalInput")
          for v in ("bq", "bk", "bv", "bo", "g0", "b0", "g1", "b1")}
    dO = nc.dram_tensor("Out", [S, D], F32, kind="ExternalOutput")

    with tile.TileContext(nc) as tc:
        with (
            tc.tile_pool(name="const", bufs=1) as cpool,
            tc.tile_pool(name="big", bufs=1) as bpool,
            tc.tile_pool(name="ut", bufs=6) as utp,
        ):
            scps = tc.alloc_tile_pool(name="scps", bufs=2, space="PSUM")
            accps = tc.alloc_tile_pool(name="accps", bufs=1, space="PSUM")
            # Pin the ACT LUT set that covers Exp+Ln+Square+Copy so the
            # table-load pass never needs to switch tables mid-kernel.
            from concourse.bacc import get_activation_tables
            _tabs = list(get_activation_tables(nc.m.arch))
            nc.scalar.add_instruction(mybir.InstLoadActFuncSet(
                name=nc.get_next_instruction_name(),
                act_func_set_id=_tabs.index("natural_log_exp_and_others"),
                ins=[], outs=[]))

            # ---------------- constants ----------------
            id128 = cpool.tile([128, 128], F32)
            masks.make_identity(nc, id128[:])
            onesf0 = cpool.tile([128, 128], F32)
            nc.vector.memset(onesf0[:], 1.0)
            # PE warmup: dummy transposes ramp the tensor engine to full
            # clock while the input DMAs are still in flight.
            with tc.tile_pool(name="warm", bufs=2, space="PSUM") as wps:
                for _i in range(8):
                    pw = wps.tile([128, 128], F32, name="pw", tag="pw")
                    nc.tensor.transpose(pw[:], onesf0[:], id128[:])
            w_r = {}
            for w in ("Wq", "Wk", "Wv", "Wo"):
                w_r[w] = cpool.tile([128, ET, D], F32R, name=f"wr_{w}", tag=f"wr_{w}")
            onesc = cpool.tile([128, 1], F32)
            nc.vector.memset(onesc[:], 1.0 / D)
            onescr = cpool.tile([128, 1], F32R)
            nc.vector.tensor_copy(onescr[:], onesc[:])
            epst = cpool.tile([1, 1], F32)
            nc.vector.memset(epst[:], EPS)

            qT = bpool.tile([128, ET, S], F32R)       # projected q, transposed
            kT = bpool.tile([128, ET, NK], F32R)      # projected k, transposed
            v_sb = bpool.tile([128, KT, H, 65], F32R)  # v [k, h, dh + ones col]
            OT = bpool.tile([128, ET, S], F32R)       # attention out + residual
            O1 = bpool.tile([128, ET, S], F32R)       # LN0 out
            O2 = bpool.tile([128, ET, S], F32R)       # FFN+residual out
            O3 = bpool.tile([128, ET, S], F32)        # LN1 out (transposed)
            On = bpool.tile([128, 8, D], F32)         # final out, natural
            nc.vector.tensor_copy(v_sb[:, :, :, 64:65], onesf0[:, 0:KT * H])

            # DMA issue order: weights (small, gate projections), Q (gates
            # the first attention block), K chunks (streamed).
            stpool = tc.alloc_tile_pool(name="stage", bufs=1)
            QT = stpool.tile([128, ET, S], F32R)      # raw Q^T
            KTr = stpool.tile([128, ET, NK], F32R)    # raw K^T
            wn = {}
            for w in ("Wq", "Wk", "Wv", "Wo"):
                wn[w] = stpool.tile([128, ET, D], F32, name=f"wn_{w}", tag=f"wn_{w}")
                nc.sync.dma_start(
                    wn[w][:], dW[w].rearrange("(et p) d -> p et d", p=128))
            qn = stpool.tile([128, 8, D], F32)
            nc.sync.dma_start(
                qn[:, 0:4, :],
                dQ.rearrange("(g st p) d -> g p st d", p=128, st=4)[0])
            kn = stpool.tile([128, KT, D], F32)
            for g in range(4):
                nc.sync.dma_start(
                    kn[:, g * 4:(g + 1) * 4, :],
                    dK.rearrange("(g st p) d -> g p st d", p=128, st=4)[g])
            nc.sync.dma_start(
                qn[:, 4:8, :],
                dQ.rearrange("(g st p) d -> g p st d", p=128, st=4)[1])
            vecs = {}
            need = {"bq": use_bq, "bk": use_bk, "bv": use_bv, "bo": use_bo,
                    "g0": use_g0, "b0": use_g0, "g1": use_g1, "b1": use_g1}
            for v in ("bq", "bk", "bv", "bo", "g0", "b0", "g1", "b1"):
                if not need[v]:
                    continue
                t = cpool.tile([128, ET], F32, name=f"vec_{v}", tag=f"vec_{v}")
                nc.sync.dma_start(t[:], dV[v].rearrange("(et e) -> e et", e=128))
                vecs[v] = t

            # ---------------- building blocks ----------------
            def attention_gen(hp, qb, acc):
                """Software-pipelined attention matmul loop; yields after
                each kt so other work can be interleaved into the stream."""
                qsl = slice(qb * 512, (qb + 1) * 512)
                uts = {}
                for kt in range(KT + 1):
                    if kt < KT:
                        sc = scps.tile([128, 1024], F32, name="sc", tag="sc")
                        for hh in range(2):
                            off = hh * 64
                            nc.tensor.matmul(
                                sc[:, hh * 512:(hh + 1) * 512],
                                kT[off:off + 64, hp, kt * 128:(kt + 1) * 128],
                                qT[off:off + 64, hp, qsl],
                                start=True, stop=True)
                        ut = utp.tile([128, 1024], F32R)
                        nc.scalar.activation(ut[:], sc[:], AF.Exp, scale=SCALE)
                        uts[kt] = ut
                    if kt >= 1:
                        utp_ = uts.pop(kt - 1)
                        for hh in range(2):
                            h = hp * 2 + hh
                            nc.tensor.matmul(
                                acc[hh][:],
                                v_sb[:, kt - 1, h, :],
                                utp_[:, hh * 512:(hh + 1) * 512],
                                start=(kt - 1 == 0), stop=(kt - 1 == KT - 1))
                    yield kt

            def new_acc(hp, qb):
                return [accps.tile([65, 512], F32, name=f"acc{hp}{qb}{_h}",
                                   tag=f"acc{_h}") for _h in range(2)]

            def acc_spill(acc, act=False):
                """Copy acc PSUM->SBUF right after the block's last matmul so
                the PSUM bank frees early (next block's A@V won't stall)."""
                accS = []
                for hh in range(2):
                    s = smp.tile([65, 512], F32, name=f"accS{hh}", tag=f"accS{hh}")
                    if act and hh == 1:
                        nc.scalar.copy(s[:], acc[hh][:])
                    else:
                        nc.vector.tensor_copy(s[:], acc[hh][:])
                    accS.append(s)
                return accS

            def attention_norm(hp, qb, accS, aoff=0, qw=512):
                qsl = slice(qb * 512 + aoff, qb * 512 + aoff + qw)
                asl = slice(aoff, aoff + qw)
                for hh in range(2):
                    den0 = smp.tile([1, 512], F32, name=f"den0{hh}", tag="den0")
                    nc.sync.dma_start(den0[0:1, 0:qw], accS[hh][64:65, asl])
                    rec = smp.tile([1, 512], F32, name=f"rec{hh}", tag="rec")
                    nc.vector.reciprocal_approx_fast(out=rec[0:1, 0:qw],
                                                     in_=den0[0:1, 0:qw])
                    recBC = smp.tile([64, 512], F32, name=f"recBC{hh}", tag="recBC")
                    nc.gpsimd.partition_broadcast(recBC[:, 0:qw], rec[0:1, 0:qw])
                    tmp = smp.tile([64, 512], F32, name=f"tmp{hh}", tag="tmp")
                    nc.vector.tensor_mul(tmp[:, 0:qw], accS[hh][0:64, asl],
                                         recBC[:, 0:qw])
                    if hh == 0:
                        nc.vector.tensor_add(OT[0:64, hp, qsl], tmp[:, 0:qw],
                                             qT[0:64, hp, qsl])
                    else:
                        tsh = smp.tile([128, 512], F32, name="tsh", tag="tsh")
                        nc.sync.dma_start(tsh[64:128, 0:qw], tmp[:, 0:qw])
                        nc.vector.tensor_add(OT[64:128, hp, qsl],
                                             tsh[64:128, 0:qw],
                                             qT[64:128, hp, qsl])
                if use_bv:
                    nc.vector.tensor_scalar_add(OT[:, hp, qsl], OT[:, hp, qsl],
                                                vecs["bv"][:, hp:hp + 1])

            def ln_stats1(x, lo, w, cps, eng, xeng=None):
                """LN stats part 1: no ACT ops on the critical stream."""
                qsl = slice(lo, lo + w)
                xsq = lnsq.tile([128, ET, w], F32R, name="xsq", tag=f"xsq{w}")
                for et in range(ET):
                    if xeng is None:
                        nc.gpsimd.tensor_mul(xsq[:, et, :], x[:, et, qsl], x[:, et, qsl])
                    elif xeng == "act":
                        nc.scalar.activation(xsq[:, et, :], x[:, et, qsl], AF.Square)
                    else:
                        nc.vector.tensor_mul(xsq[:, et, :], x[:, et, qsl], x[:, et, qsl])
                mus = cps.tile([1, w], F32, name="mus", tag="c")
                sqs = cps.tile([1, w], F32, name="sqs", tag="c")
                for et in range(ET):
                    nc.tensor.matmul(mus[:], onescr[:], x[:, et, qsl],
                                     start=(et == 0), stop=(et == ET - 1))
                    nc.tensor.matmul(sqs[:], onescr[:], xsq[:, et, :],
                                     start=(et == 0), stop=(et == ET - 1))
                mu0 = lnsm.tile([1, w], F32, name="mu0", tag="mu0")
                eng.tensor_copy(mu0[:], mus[:])
                musq = lnsm.tile([1, w], F32, name="musq", tag="musq")
                eng.tensor_mul(musq[:], mu0[:], mu0[:])
                var = lnsm.tile([1, w], F32, name="var", tag="var")
                eng.tensor_sub(var[:], sqs[:], musq[:])
                return mu0, var

            def ln_stats2(mu0, var, w):
                """LN stats part 2: the two ACT LUT ops + row broadcasts.
                Emit well after part 1 so the ACT stream never stalls."""
                lnv = lnsm.tile([1, w], F32, name="lnv", tag="lnv")
                nc.scalar.activation(lnv[:], var[:], AF.Ln, bias=epst[:])
                rst = lnsm.tile([1, w], F32, name="rst", tag="rst")
                nc.scalar.activation(rst[:], lnv[:], AF.Exp, scale=-0.5)
                muB = lnsm.tile([128, w], F32, name="muB", tag="muB")
                nc.gpsimd.partition_broadcast(muB[:], mu0[0:1, :])
                rsB = lnsm.tile([128, w], F32, name="rsB", tag="rsB")
                nc.gpsimd.partition_broadcast(rsB[:], rst[0:1, :])
                return muB, rsB

            def ln_apply(x, y, lo, w, muB, rsB, gname, bname, use_g, flip=0):
                qsl = slice(lo, lo + w)
                for et in range(ET):
                    eng = nc.vector if (et + flip) % 2 == 0 else nc.gpsimd
                    cen = lnsm.tile([128, w], F32, name="cen", tag="cen")
                    eng.tensor_sub(cen[:], x[:, et, qsl], muB[:])
                    dst = y[:, et, qsl]
                    eng.tensor_mul(dst, cen[:], rsB[:])
                    if use_g:
                        nc.vector.tensor_scalar(
                            dst, dst, vecs[gname][:, et:et + 1],
                            vecs[bname][:, et:et + 1], ALU.mult, ALU.add)

            def ffn(lo, w, cps):
                qsl = slice(lo, lo + w)
                for et in range(ET):
                    ps = cps.tile([128, w], F32, name="ffps", tag="c")
                    for dt in range(ET):
                        nc.tensor.matmul(
                            ps[:], w_r["Wo"][:, dt, et * 128:(et + 1) * 128],
                            O1[:, dt, qsl],
                            start=(dt == 0), stop=(dt == ET - 1))
                    if use_bo:
                        ft = lnsm.tile([128, w], F32, name="ft", tag="ft")
                        nc.vector.tensor_scalar(
                            ft[:], ps[:], vecs["bo"][:, et:et + 1],
                            0.0, ALU.add, ALU.max)
                        nc.gpsimd.tensor_add(O2[:, et, qsl], O1[:, et, qsl], ft[:])
                    else:
                        nc.vector.scalar_tensor_tensor(
                            O2[:, et, qsl], ps[:], 0.0, O1[:, et, qsl],
                            ALU.max, ALU.add)

            def store(lo, nst, cps, act=False):
                for st in range(nst):
                    po = cps.tile([128, 256], F32, name="po", tag="c")
                    for dt in range(ET):
                        nc.tensor.transpose(
                            po[:, dt * 128:(dt + 1) * 128],
                            O3[:, dt, lo + st * 128:lo + (st + 1) * 128],
                            id128[:])
                    (nc.scalar.copy if act and st % 2 else nc.vector.tensor_copy)(
                        On[:, lo // 128 + st, :], po[:])
                g2 = lo // 256
                for h in range(nst // 2):
                    nc.sync.dma_start(
                        dO.rearrange("(g st p) d -> g p st d", p=128, st=2)[g2 + h],
                        On[:, (g2 + h) * 2:(g2 + h + 1) * 2, :])

            def drive(gen, n):
                for _ in range(n):
                    next(gen, None)

            # ------- stages 0-3: load/transpose/project + att(q0,h0) -------
            with tc.tile_pool(name="pa", bufs=2, space="PSUM") as pa:
                for w in ("Wq", "Wk", "Wv", "Wo"):
                    for dt in range(ET):
                        ps = pa.tile([128, 512], F32, name="pw2", tag="pa")
                        for et in range(ET):
                            nc.tensor.transpose(
                                ps[:, et * 128:(et + 1) * 128],
                                wn[w][:, et, dt * 128:(dt + 1) * 128], id128[:])
                        (nc.vector.tensor_copy if dt == 0 else nc.scalar.copy)(
                            w_r[w][:, dt, :], ps[:, 0:D])
                def q_chunk(g, pool=None, tag="pa"):   # Q: transpose + q-proj
                    pool = pool or pa
                    for dt in range(ET):
                        ps = pool.tile([128, 512], F32, name="pq", tag=tag)
                        for j in range(4):
                            nc.tensor.transpose(
                                ps[:, j * 128:(j + 1) * 128],
                                qn[:, g * 4 + j, dt * 128:(dt + 1) * 128], id128[:])
                        nc.vector.tensor_copy(
                            QT[:, dt, g * 512:(g + 1) * 512], ps[:])
                    for et in range(ET):
                        ps = pool.tile([128, 512], F32, name="pq2", tag=tag)
                        for dt in range(ET):
                            nc.tensor.matmul(
                                ps[:], w_r["Wq"][:, dt, et * 128:(et + 1) * 128],
                                QT[:, dt, g * 512:(g + 1) * 512],
                                start=(dt == 0), stop=(dt == ET - 1))
                        dst = qT[:, et, g * 512:(g + 1) * 512]
                        if use_bq:
                            nc.vector.tensor_scalar_add(dst, ps[:], vecs["bq"][:, et:et + 1])
                        else:
                            nc.vector.tensor_copy(dst, ps[:])

                q_chunk(0)
                acc00 = new_acc(0, 0)
                g00 = attention_gen(0, 0, acc00)
                for g in range(4):          # K chunks: transpose + k/v-proj
                    for dt in range(ET):
                        ps = pa.tile([128, 512], F32, name="pk", tag="pa")
                        for j in range(4):
                            nc.tensor.transpose(
                                ps[:, j * 128:(j + 1) * 128],
                                kn[:, g * 4 + j, dt * 128:(dt + 1) * 128], id128[:])
                        nc.vector.tensor_copy(
                            KTr[:, dt, g * 512:(g + 1) * 512], ps[:])
                    for et in range(ET):
                        ps = pa.tile([128, 512], F32, name="pk2", tag="pa")
                        for dt in range(ET):
                            nc.tensor.matmul(
                                ps[:], w_r["Wk"][:, dt, et * 128:(et + 1) * 128],
                                KTr[:, dt, g * 512:(g + 1) * 512],
                                start=(dt == 0), stop=(dt == ET - 1))
                        dst = kT[:, et, g * 512:(g + 1) * 512]
                        if use_bk:
                            nc.vector.tensor_scalar_add(dst, ps[:], vecs["bk"][:, et:et + 1])
                        else:
                            nc.vector.tensor_copy(dst, ps[:])
                    for j in range(4):
                        kt = g * 4 + j
                        ps = pa.tile([128, 512], F32, name="pv", tag="pa")
                        for dt in range(ET):
                            nc.tensor.matmul(
                                ps[:, 0:256], KTr[:, dt, kt * 128:(kt + 1) * 128],
                                w_r["Wv"][:, dt, :],
                                start=(dt == 0), stop=(dt == ET - 1))
                        nc.vector.tensor_copy(v_sb[:, kt, :, 0:64], ps[:, 0:256])
                    drive(g00, 2)           # attention(q0,h0) partial overlap

            # ------- stages 4-7: attention blocks + phase C overlap -------
            smp = tc.alloc_tile_pool(name="sm", bufs=2)
            cps = tc.alloc_tile_pool(name="cps", bufs=2, space="PSUM")
            if True:
                drive(g00, KT + 1)          # finish att(q0,h0)
                accS00 = acc_spill(acc00)

                acc01 = new_acc(1, 0)
                g01 = attention_gen(1, 0, acc01)
                drive(g01, 2)
                attention_norm(0, 0, accS00)
                drive(g01, 8)
                q_chunk(1, cps, "c")        # q1 projection in block-2 PE slack
                stpool.release()
                lnsm = tc.alloc_tile_pool(name="lnsm", bufs=3)
                lnsq = tc.alloc_tile_pool(name="lnsq", bufs=3)
                drive(g01, KT + 1)
                accS01 = acc_spill(acc01)

                acc10 = new_acc(0, 1)
                g10 = attention_gen(0, 1, acc10)
                drive(g10, 2)
                attention_norm(1, 0, accS01)    # completes OT(q0)
                drive(g10, 3)
                mv0 = ln_stats1(OT, 0, 512, cps, nc.vector)   # LN0(q0)
                drive(g10, 3)
                mb0 = ln_stats2(*mv0, 512)
                drive(g10, 3)
                ln_apply(OT, O1, 0, 512, *mb0, "g0", "b0", use_g0)
                drive(g10, KT + 1)
                accS10 = acc_spill(acc10)

                acc11 = new_acc(1, 1)
                g11 = attention_gen(1, 1, acc11)
                drive(g11, 2)
                attention_norm(0, 1, accS10)
                drive(g11, 2)
                ffn(0, 512, cps)                 # FFN(q0)
                drive(g11, 3)
                mv1 = ln_stats1(O2, 0, 512, cps, nc.vector)   # LN1(q0)
                drive(g11, 3)
                mb1 = ln_stats2(*mv1, 512)
                drive(g11, 3)
                ln_apply(O2, O3, 0, 512, *mb1, "g1", "b1", use_g1)
                drive(g11, 3)
                store(0, 4, cps)
                drive(g11, KT + 1)
                accS11 = acc_spill(acc11, act=True)

                cps.release()
                accps.release()
                scps.release()
                cps = tc.alloc_tile_pool(name="tailps", bufs=6, space="PSUM")

                # tail: two 256-wide pipelined chains per stage, with the
                # softmax normalization itself split so chain A starts early
                attention_norm(1, 1, accS11, 0, 256)
                mvA = ln_stats1(OT, 512, 256, cps, nc.vector, "act")
                attention_norm(1, 1, accS11, 256, 256)
                mbA = ln_stats2(*mvA, 256)
                mvB = ln_stats1(OT, 768, 256, cps, nc.vector, "dve")
                ln_apply(OT, O1, 512, 256, *mbA, "g0", "b0", use_g0)
                mbB = ln_stats2(*mvB, 256)
                ffn(512, 256, cps)
                ln_apply(OT, O1, 768, 256, *mbB, "g0", "b0", use_g0, flip=1)
                mvC = ln_stats1(O2, 512, 256, cps, nc.vector, "act")
                ffn(768, 256, cps)
                mbC = ln_stats2(*mvC, 256)
                mvD = ln_stats1(O2, 768, 256, cps, nc.vector, "dve")
                ln_apply(O2, O3, 512, 256, *mbC, "g1", "b1", use_g1)
                mbD = ln_stats2(*mvD, 256)
                store(512, 2, cps, act=True)
                ln_apply(O2, O3, 768, 256, *mbD, "g1", "b1", use_g1, flip=1)
                store(768, 2, cps, act=True)
            cps.release()
            lnsq.release()
            lnsm.release()
            smp.release()

    nc.compile()
    return nc


def kernel(Q, K, Wq, bq, Wk, bk, Wv, bv, Wo, bo, g0, b0, g1, b1):
    Q, K = np.asarray(Q), np.asarray(K)
    ws = {n: np.ascontiguousarray(np.asarray(v), dtype=np.float32)
          for n, v in (("Wq", Wq), ("Wk", Wk), ("Wv", Wv), ("Wo", Wo))}
    vs = {n: np.ascontiguousarray(np.asarray(v), dtype=np.float32)
          for n, v in (("bq", bq), ("bk", bk), ("bv", bv), ("bo", bo),
                       ("g0", g0), ("b0", b0), ("g1", g1), ("b1", b1))}
    flags = (bool(np.any(vs["bq"])), bool(np.any(vs["bk"])),
             bool(np.any(vs["bv"])), bool(np.any(vs["bo"])),
             bool(np.any(vs["g0"] != 1.0) or np.any(vs["b0"])),
             bool(np.any(vs["g1"] != 1.0) or np.any(vs["b1"])))
    if flags not in _CACHE:
        _CACHE[flags] = _build(flags)
    nc = _CACHE[flags]

    in_maps = []
    for b in range(B):
        for half in range(2):
            m = {"Qs": np.ascontiguousarray(Q[b, half * S:(half + 1) * S], dtype=np.float32),
                 "Ks": np.ascontiguousarray(K[b], dtype=np.float32)}
            m.update(ws)
            m.update(vs)
            in_maps.append(m)

    res = run_bass_kernel_spmd(nc, in_maps, list(range(8)))
    out = np.empty((B, NQ, D), dtype=np.float32)
    for i in range(8):
        b, half = divmod(i, 2)
        out[b, half * S:(half + 1) * S] = res.results[i]["Out"]
    return out
